# revision 13
# baseline (speedup 1.0000x reference)
"""Sparse-attention Trainium2 kernel (nn_AttentionLayer, B=16 S=2048 D=128).

reference semantics:
    A = Q @ T^T                     # [B,S,S]
    A = where(A > 0.3, A, 0)
    A += where(strictly_upper, -2^32, 0)
    y = softmax(A / sqrt(D)) @ V

Sharding: data-parallel over batch, 2 batches per core on 8 NeuronCores.

Per-core algorithm (per batch), v4:
  - Q, T cast to bf16 on VectorE into per-chunk staging tiles (separate
    tensors so the DMA xbar transposes' coarse-grained read deps don't
    serialize), transposed SBUF->SBUF by 3 large xbar calls per batch.
    Load DMAs are issued from sync/scalar/vector queues in parallel.
  - Scores computed transposed, S^T[k,q], 2 ktiles per [128,1024] PSUM
    tile. Straddling-diagonal k-tiles skip dead query columns and are
    left-packed in their PSUM bank so exp spans merge.
  - num = max(exp(S^T*scale),1): ScalarE exp (fp32 PSUM -> bf16 SBUF),
    VectorE tensor_scalar_max (4x mode). The causal mask of each
    diagonal 128x128 block is a GpSimd multiply with a 0/1 triangle.
  - PV + denominator fused per (ktile, q-subtile): lhsT = num chunk,
    rhs = [V | ones] [128k,129], PSUM-accumulated; obanks packed two
    per PSUM bank (only the bank's first matmul sets start: start=True
    arms a bank-wide lazy zero).
  - out = PV/den via a VectorE PSUM->SBUF copy + GpSimd normalize_recip.
  - PE warm-up matmuls ramp the p-state during the DMA prep; PV for
    group g is emitted two QK groups later (software pipelining).
"""

from collections import deque
from contextlib import ExitStack

import numpy as np

import concourse.bass as bass
import concourse.mybir as mybir
import concourse.tile as tile
from concourse import bacc

B, S, D = 16, 2048, 128
N_CORES = 8
B_LOC = B // N_CORES
QB = 512
KT = 128
N_QB = S // QB
N_ST = S // 128
SCALE = float(1.0 / np.sqrt(D))

F32 = mybir.dt.float32
BF16 = mybir.dt.bfloat16
Alu = mybir.AluOpType


def build_attention_core():
    nc = bacc.Bacc("TRN2", target_bir_lowering=False, debug=False,
                   num_devices=N_CORES)
    q_ext = nc.dram_tensor("Q", [B_LOC, S, D], F32, kind="ExternalInput").ap()
    t_ext = nc.dram_tensor("T", [B_LOC, S, D], F32, kind="ExternalInput").ap()
    v_ext = nc.dram_tensor("V", [B_LOC, S, D], F32, kind="ExternalInput").ap()
    o_ext = nc.dram_tensor("out", [B_LOC, S, D], F32, kind="ExternalOutput").ap()

    with tile.TileContext(nc) as tc, ExitStack() as ctx:
        const_pool = ctx.enter_context(tc.tile_pool(name="const", bufs=1))
        nat_pool = ctx.enter_context(tc.tile_pool(name="nat", bufs=1))
        stage_pool = ctx.enter_context(tc.tile_pool(name="stage", bufs=1))
        tpd_pool = ctx.enter_context(tc.tile_pool(name="tpd", bufs=1))
        vb_pool = ctx.enter_context(tc.tile_pool(name="vb", bufs=1))
        num_pool = ctx.enter_context(tc.tile_pool(name="num", bufs=6))
        fin_pool = ctx.enter_context(tc.tile_pool(name="fin", bufs=2))
        rec_pool = ctx.enter_context(tc.tile_pool(name="rec", bufs=4))
        qk_psum = ctx.enter_context(tc.tile_pool(name="qk_ps", bufs=2, space="PSUM"))
        ob_psum = ctx.enter_context(tc.tile_pool(name="ob_ps", bufs=4, space="PSUM"))

        # ---- constants (gpsimd) ----
        junk = const_pool.tile([128, 512], BF16, name="junk")
        nc.gpsimd.memset(junk[:], 0.25)
        # tri01[p, n] = 0 if p > n else 1 (first 128 cols form the in-tile
        # causal keep-mask; cols >= 128 are all ones)
        tri01 = const_pool.tile([128, 128], BF16, name="tri01")
        nc.gpsimd.memset(tri01[:], 1.0)
        nc.gpsimd.affine_select(
            out=tri01[:], in_=tri01[:],
            compare_op=Alu.is_ge, fill=0.0,
            base=0, channel_multiplier=-1, pattern=[[1, 128]])

        # ---- PE warm-up: ramp the p-state while DMA prep runs ----
        for w in range(12):
            wps = qk_psum.tile([128, 1024], F32, tag="qk", name=f"wps{w}")
            nc.tensor.matmul(wps[:, 0:512], lhsT=junk[:, 0:128], rhs=junk[:])

        # ---- staging: per batch, 3 chunk tiles (separate tensors so the
        # xbar transpose read-dep doesn't wait on later casts) ----
        # chunk c0: [q0:4 | t0:4]; chunk cQ: q4:16; chunk cT: t4:16
        nats, stages, qt_tps, v_augs = [], [], [], []
        for b in range(B_LOC):
            q_nat = nat_pool.tile([128, N_ST, D], F32, name=f"qnat{b}")
            t_nat = nat_pool.tile([128, N_ST, D], F32, name=f"tnat{b}")
            v_nat = nat_pool.tile([128, N_ST, D], F32, name=f"vnat{b}")
            c0 = stage_pool.tile([128, 8, 128], BF16, name=f"c0_{b}")
            cq = stage_pool.tile([128, 12, 128], BF16, name=f"cq_{b}")
            ct = stage_pool.tile([128, 12, 128], BF16, name=f"ct_{b}")
            qt_tp = tpd_pool.tile([128, 2 * N_ST, 128], BF16, name=f"qttp{b}")
            v_aug = vb_pool.tile([128, N_ST, 129], BF16, name=f"vaug{b}")
            nats.append((q_nat, t_nat, v_nat))
            stages.append((c0, cq, ct))
            qt_tps.append(qt_tp); v_augs.append(v_aug)

        # transposed slot layout (matches [c0 | cQ | cT] order):
        #   q tile t: slot t if t < 4 else 8 + (t - 4)
        #   t tile c: slot 4 + c if c < 4 else 20 + (c - 4)
        def q_slot(t):
            return t if t < 4 else 8 + (t - 4)

        def t_slot(c):
            return 4 + c if c < 4 else 20 + (c - 4)

        def load(b, which, h):
            q_nat, t_nat, v_nat = nats[b]
            nat = {"q": q_nat, "t": t_nat, "v": v_nat}[which]
            ext = {"q": q_ext, "t": t_ext, "v": v_ext}[which]
            eng = {"q": nc.sync, "t": nc.sync, "v": nc.sync}[which]
            if which == "q" and h == 1:
                eng = nc.scalar
            if which == "t" and h == 1:
                eng = nc.gpsimd
            sl = slice(0, 4) if h == 0 else slice(4, 16)
            ssl = slice(0, 512) if h == 0 else slice(512, 2048)
            eng.dma_start(
                nat[:, sl, :],
                ext[b, ssl, :].rearrange("(t p) d -> p t d", p=128))

        def cast_c0(b):
            q_nat, t_nat, v_nat = nats[b]
            c0 = stages[b][0]
            nc.vector.tensor_copy(c0[:, 0:4, :], q_nat[:, 0:4, :])
            nc.vector.tensor_copy(c0[:, 4:8, :], t_nat[:, 0:4, :])

        def cast_big(b, which, h):
            """Cast half of q4:16 (which='q') or t4:16 into cQ/cT."""
            q_nat, t_nat, v_nat = nats[b]
            nat = q_nat if which == "q" else t_nat
            stg = stages[b][1] if which == "q" else stages[b][2]
            sl = slice(0, 6) if h == 0 else slice(6, 12)
            nsl = slice(4, 10) if h == 0 else slice(10, 16)
            nc.vector.tensor_copy(stg[:, sl, :], nat[:, nsl, :])

        def cast_v(b, h):
            q_nat, t_nat, v_nat = nats[b]
            sl = slice(0, 4) if h == 0 else slice(4, 16)
            nc.gpsimd.memset(v_augs[b][:, sl, D:D + 1], 1.0)
            nc.vector.tensor_copy(v_augs[b][:, sl, 0:D], v_nat[:, sl, :])

        def transpose(b, chunk):
            src = stages[b][chunk]
            lo = (0, 8, 20)[chunk]
            n = (8, 12, 12)[chunk]
            nc.sync.dma_start_transpose(
                qt_tps[b][:, lo:lo + n, :],
                src[:].rearrange("p t d -> p (t d)"))

        # ---- batch-0 head ----
        load(0, "q", 0)
        load(0, "t", 0)
        load(0, "v", 0)
        load(0, "q", 1)     # scalar queue
        load(0, "t", 1)     # vector queue (issued before any DVE cast)
        cast_c0(0)
        transpose(0, 0)
        cast_v(0, 0)
        cast_big(0, "q", 0)
        cast_big(0, "q", 1)
        cast_big(0, "t", 0)
        cast_big(0, "t", 1)

        items = []
        for b in range(B_LOC):
            for qb in range(N_QB):
                for g in range((4 * qb + 4) // 2):
                    items.append((b, qb, g))

        prep_at = {
            0: lambda: transpose(0, 1),
            1: lambda: transpose(0, 2),
            2: lambda: (load(0, "v", 1), cast_v(0, 1)),
            4: lambda: (load(1, "q", 0), load(1, "t", 0), load(1, "v", 0)),
            5: lambda: (load(1, "q", 1), load(1, "t", 1)),
            6: lambda: cast_c0(1),
            7: lambda: (transpose(1, 0), cast_big(1, "q", 0)),
            8: lambda: cast_big(1, "q", 1),
            9: lambda: (transpose(1, 1), cast_big(1, "t", 0)),
            10: lambda: cast_big(1, "t", 1),
            11: lambda: (transpose(1, 2), load(1, "v", 1), cast_v(1, 0)),
            13: lambda: cast_v(1, 1),
        }

        state = {}

        def qk_group(b, qb, g):
            q0 = qb * QB
            qt_tp = qt_tps[b]
            s_ps = qk_psum.tile([128, 1024], F32, tag="qk")
            num = num_pool.tile([128, 1024], BF16, tag="num")
            act_spans = []      # merged contiguous spans (left-packed)
            mask_blocks = []    # span starts of diagonal blocks
            for j, c in enumerate((2 * g, 2 * g + 1)):
                i = c - 4 * qb
                lo = 128 * i if i > 0 else 0
                w = QB - lo
                ql = q0 + lo
                t0_ = ql // 128
                nt = (QB - lo) // 128
                rhs = qt_tp[:, q_slot(t0_):q_slot(t0_) + nt, :] \
                    .rearrange("p t q -> p (t q)")
                nc.tensor.matmul(
                    s_ps[:, j * 512:j * 512 + w],
                    lhsT=qt_tp[:, t_slot(c), :],
                    rhs=rhs,
                )
                if act_spans and act_spans[-1][1] == j * 512:
                    act_spans[-1] = (act_spans[-1][0], j * 512 + w)
                else:
                    act_spans.append((j * 512, j * 512 + w))
                if i >= 0:
                    mask_blocks.append(j * 512)
            for lo_, hi_ in act_spans:
                nc.scalar.activation(num[:, lo_:hi_], s_ps[:, lo_:hi_],
                                     mybir.ActivationFunctionType.Exp,
                                     scale=SCALE)
                nc.vector.tensor_scalar_max(num[:, lo_:hi_],
                                            num[:, lo_:hi_], 1.0)
            for ds in mask_blocks:
                nc.gpsimd.tensor_tensor(num[:, ds:ds + 128],
                                        num[:, ds:ds + 128], tri01[:],
                                        op=Alu.mult)
            st = state.setdefault((b, qb), {"ob": None, "num": {}})
            if st["ob"] is None:
                st["ob"] = [ob_psum.tile([128, 2, 256], F32, tag="ob",
                                         name=f"ob_{b}_{qb}_{h}")
                            for h in range(2)]
            st["num"][g] = num

        def pv_group(b, qb, g):
            st = state[(b, qb)]
            num = st["num"].pop(g)
            v_aug = v_augs[b]
            for j, c in enumerate((2 * g, 2 * g + 1)):
                i = c - 4 * qb
                lo = 128 * i if i > 0 else 0
                for sub in range(max(i, 0), 4):
                    ob = st["ob"][sub // 2]
                    nc.tensor.matmul(
                        ob[:, sub % 2, 0:129],
                        lhsT=num[:, j * 512 + sub * 128 - lo:
                                 j * 512 + (sub + 1) * 128 - lo],
                        rhs=v_aug[:, c, 0:129],
                        start=(c == 0 and sub % 2 == 0),
                        stop=(c == 4 * qb + sub),
                        skip_group_check=True,
                    )

        def finalize(b, qb):
            st = state.pop((b, qb))
            o_tile = fin_pool.tile([128, 4, 128], F32, tag="fin")
            for h in range(2):
                ob_sb = rec_pool.tile([128, 2, 129], F32, tag="rec")
                nc.vector.tensor_copy(ob_sb[:], st["ob"][h][:, :, 0:129])
                for s2 in range(2):
                    nc.gpsimd.normalize_recip(
                        o_tile[:, 2 * h + s2, :],
                        ob_sb[:, s2, 0:128],
                        ob_sb[:, s2, 128:129])
            nc.sync.dma_start(
                o_ext[b, qb * QB:(qb + 1) * QB, :]
                    .rearrange("(s p) d -> p s d", p=128),
                o_tile[:])

        pending = deque()

        def flush_one():
            b, qb, g = pending.popleft()
            pv_group(b, qb, g)
            if g == (4 * qb + 4) // 2 - 1:
                finalize(b, qb)

        for idx, it in enumerate(items):
            qk_group(*it)
            if idx in prep_at:
                prep_at[idx]()
            pending.append(it)
            if len(pending) > 2:
                flush_one()
        while pending:
            flush_one()

    nc.compile()
    return nc


_NC_CACHE = None


def _get_nc():
    global _NC_CACHE
    if _NC_CACHE is None:
        _NC_CACHE = build_attention_core()
    return _NC_CACHE


def kernel(Q: np.ndarray, T: np.ndarray, V: np.ndarray) -> np.ndarray:
    """Full-input entry point: shard over batch, run 8-core SPMD, gather."""
    from concourse.bass_utils import run_bass_kernel_spmd

    Q = np.ascontiguousarray(np.asarray(Q, dtype=np.float32))
    T = np.ascontiguousarray(np.asarray(T, dtype=np.float32))
    V = np.ascontiguousarray(np.asarray(V, dtype=np.float32))
    assert Q.shape == (B, S, D), Q.shape

    nc = _get_nc()
    in_maps = [
        {
            "Q": Q[i * B_LOC:(i + 1) * B_LOC],
            "T": T[i * B_LOC:(i + 1) * B_LOC],
            "V": V[i * B_LOC:(i + 1) * B_LOC],
        }
        for i in range(N_CORES)
    ]
    res = run_bass_kernel_spmd(nc, in_maps, core_ids=list(range(N_CORES)))
    return np.concatenate([res.results[i]["out"] for i in range(N_CORES)], axis=0)


# revision 14
# speedup vs baseline: 1.8221x; 1.8221x over previous
"""Sparse-attention Trainium2 kernel (nn_AttentionLayer, B=16 S=2048 D=128).

reference semantics:
    A = Q @ T^T                     # [B,S,S]
    A = where(A > 0.3, A, 0)
    A += where(strictly_upper, -2^32, 0)
    y = softmax(A / sqrt(D)) @ V

Sharding: data-parallel over batch, 2 batches per core on 8 NeuronCores.

Per-core algorithm (per batch), v4:
  - Q, T cast to bf16 on VectorE into per-chunk staging tiles (separate
    tensors so the DMA xbar transposes' coarse-grained read deps don't
    serialize), transposed SBUF->SBUF by 3 large xbar calls per batch.
    Load DMAs are issued from sync/scalar/vector queues in parallel.
  - Scores computed transposed, S^T[k,q], 2 ktiles per [128,1024] PSUM
    tile. Straddling-diagonal k-tiles skip dead query columns and are
    left-packed in their PSUM bank so exp spans merge.
  - num = max(exp(S^T*scale),1): ScalarE exp (fp32 PSUM -> bf16 SBUF),
    VectorE tensor_scalar_max (4x mode). The causal mask of each
    diagonal 128x128 block is a GpSimd multiply with a 0/1 triangle.
  - PV + denominator fused per (ktile, q-subtile): lhsT = num chunk,
    rhs = [V | ones] [128k,129], PSUM-accumulated; obanks packed two
    per PSUM bank (only the bank's first matmul sets start: start=True
    arms a bank-wide lazy zero).
  - out = PV/den via a VectorE PSUM->SBUF copy + GpSimd normalize_recip.
  - PE warm-up matmuls ramp the p-state during the DMA prep; PV for
    group g is emitted two QK groups later (software pipelining).
"""

from collections import deque
from contextlib import ExitStack

import numpy as np

import concourse.bass as bass
import concourse.mybir as mybir
import concourse.tile as tile
from concourse import bacc

B, S, D = 16, 2048, 128
N_CORES = 8
B_LOC = B // N_CORES
QB = 512
KT = 128
N_QB = S // QB
N_ST = S // 128
SCALE = float(1.0 / np.sqrt(D))

F32 = mybir.dt.float32
BF16 = mybir.dt.bfloat16
Alu = mybir.AluOpType


def build_attention_core():
    nc = bacc.Bacc("TRN2", target_bir_lowering=False, debug=False,
                   num_devices=N_CORES)
    q_ext = nc.dram_tensor("Q", [B_LOC, S, D], F32, kind="ExternalInput").ap()
    t_ext = nc.dram_tensor("T", [B_LOC, S, D], F32, kind="ExternalInput").ap()
    v_ext = nc.dram_tensor("V", [B_LOC, S, D], F32, kind="ExternalInput").ap()
    o_ext = nc.dram_tensor("out", [B_LOC, S, D], F32, kind="ExternalOutput").ap()

    with tile.TileContext(nc) as tc, ExitStack() as ctx:
        const_pool = ctx.enter_context(tc.tile_pool(name="const", bufs=1))
        nat_pool = ctx.enter_context(tc.tile_pool(name="nat", bufs=1))
        stage_pool = ctx.enter_context(tc.tile_pool(name="stage", bufs=1))
        tpd_pool = ctx.enter_context(tc.tile_pool(name="tpd", bufs=1))
        vb_pool = ctx.enter_context(tc.tile_pool(name="vb", bufs=1))
        num_pool = ctx.enter_context(tc.tile_pool(name="num", bufs=6))
        fin_pool = ctx.enter_context(tc.tile_pool(name="fin", bufs=2))
        rec_pool = ctx.enter_context(tc.tile_pool(name="rec", bufs=4))
        qk_psum = ctx.enter_context(tc.tile_pool(name="qk_ps", bufs=2, space="PSUM"))
        ob_psum = ctx.enter_context(tc.tile_pool(name="ob_ps", bufs=4, space="PSUM"))

        # ---- constants (gpsimd) ----
        junk = const_pool.tile([128, 512], BF16, name="junk")
        nc.gpsimd.memset(junk[:], 0.25)
        # tri01[p, n] = 0 if p > n else 1 (first 128 cols form the in-tile
        # causal keep-mask; cols >= 128 are all ones)
        tri01 = const_pool.tile([128, 128], BF16, name="tri01")
        nc.gpsimd.memset(tri01[:], 1.0)
        nc.gpsimd.affine_select(
            out=tri01[:], in_=tri01[:],
            compare_op=Alu.is_ge, fill=0.0,
            base=0, channel_multiplier=-1, pattern=[[1, 128]])

        # ---- PE warm-up: ramp the p-state while DMA prep runs ----
        for w in range(16):
            wps = qk_psum.tile([128, 1024], F32, tag="qk", name=f"wps{w}")
            nc.tensor.matmul(wps[:, 0:512], lhsT=junk[:, 0:128], rhs=junk[:])

        # ---- staging: per batch, 3 chunk tiles (separate tensors so the
        # xbar transpose read-dep doesn't wait on later casts) ----
        # chunk c0: [q0:4 | t0:4]; chunk cQ: q4:16; chunk cT: t4:16
        nats, stages, qt_tps, v_augs = [], [], [], []
        for b in range(B_LOC):
            q_nat = nat_pool.tile([128, N_ST, D], F32, name=f"qnat{b}")
            t_nat = nat_pool.tile([128, N_ST, D], F32, name=f"tnat{b}")
            v_nat = nat_pool.tile([128, N_ST, D], F32, name=f"vnat{b}")
            c0 = stage_pool.tile([128, 8, 128], BF16, name=f"c0_{b}")
            cq = stage_pool.tile([128, 12, 128], BF16, name=f"cq_{b}")
            ct = stage_pool.tile([128, 12, 128], BF16, name=f"ct_{b}")
            qt_tp = tpd_pool.tile([128, 2 * N_ST, 128], BF16, name=f"qttp{b}")
            v_aug = vb_pool.tile([128, N_ST, 129], BF16, name=f"vaug{b}")
            nats.append((q_nat, t_nat, v_nat))
            stages.append((c0, cq, ct))
            qt_tps.append(qt_tp); v_augs.append(v_aug)

        # transposed slot layout (matches [c0 | cQ | cT] order):
        #   q tile t: slot t if t < 4 else 8 + (t - 4)
        #   t tile c: slot 4 + c if c < 4 else 20 + (c - 4)
        def q_slot(t):
            return t if t < 4 else 8 + (t - 4)

        def t_slot(c):
            return 4 + c if c < 4 else 20 + (c - 4)

        def load(b, which, h):
            q_nat, t_nat, v_nat = nats[b]
            nat = {"q": q_nat, "t": t_nat, "v": v_nat}[which]
            ext = {"q": q_ext, "t": t_ext, "v": v_ext}[which]
            eng = {"q": nc.sync, "t": nc.sync, "v": nc.sync}[which]
            if which == "q" and h == 1:
                eng = nc.scalar
            if which == "t" and h == 1:
                eng = nc.scalar
            sl = slice(0, 4) if h == 0 else slice(4, 16)
            ssl = slice(0, 512) if h == 0 else slice(512, 2048)
            eng.dma_start(
                nat[:, sl, :],
                ext[b, ssl, :].rearrange("(t p) d -> p t d", p=128))

        def cast_c0(b):
            q_nat, t_nat, v_nat = nats[b]
            c0 = stages[b][0]
            nc.vector.tensor_copy(c0[:, 0:4, :], q_nat[:, 0:4, :])
            nc.vector.tensor_copy(c0[:, 4:8, :], t_nat[:, 0:4, :])

        def cast_big(b, which, h):
            """Cast half of q4:16 (which='q') or t4:16 into cQ/cT."""
            q_nat, t_nat, v_nat = nats[b]
            nat = q_nat if which == "q" else t_nat
            stg = stages[b][1] if which == "q" else stages[b][2]
            sl = slice(0, 6) if h == 0 else slice(6, 12)
            nsl = slice(4, 10) if h == 0 else slice(10, 16)
            nc.vector.tensor_copy(stg[:, sl, :], nat[:, nsl, :])

        def cast_v(b, h):
            q_nat, t_nat, v_nat = nats[b]
            sl = slice(0, 4) if h == 0 else slice(4, 16)
            nc.vector.memset(v_augs[b][:, sl, D:D + 1], 1.0)
            nc.vector.tensor_copy(v_augs[b][:, sl, 0:D], v_nat[:, sl, :])

        def transpose(b, chunk):
            src = stages[b][chunk]
            lo = (0, 8, 20)[chunk]
            n = (8, 12, 12)[chunk]
            nc.sync.dma_start_transpose(
                qt_tps[b][:, lo:lo + n, :],
                src[:].rearrange("p t d -> p (t d)"))

        # ---- batch-0 head ----
        load(0, "q", 0)
        load(0, "t", 0)
        load(0, "v", 0)
        load(0, "q", 1)     # scalar queue
        load(0, "t", 1)     # vector queue (issued before any DVE cast)
        cast_c0(0)
        transpose(0, 0)
        cast_v(0, 0)
        cast_big(0, "q", 0)
        cast_big(0, "q", 1)
        cast_big(0, "t", 0)
        cast_big(0, "t", 1)

        items = []
        for b in range(B_LOC):
            for qb in range(N_QB):
                for g in range((4 * qb + 4) // 2):
                    items.append((b, qb, g))

        prep_at = {
            0: lambda: transpose(0, 1),
            1: lambda: transpose(0, 2),
            2: lambda: (load(0, "v", 1), cast_v(0, 1)),
            4: lambda: (load(1, "q", 0), load(1, "t", 0), load(1, "v", 0)),
            5: lambda: (load(1, "q", 1), load(1, "t", 1)),
            6: lambda: cast_c0(1),
            7: lambda: (transpose(1, 0), cast_big(1, "q", 0)),
            8: lambda: cast_big(1, "q", 1),
            9: lambda: (transpose(1, 1), cast_big(1, "t", 0)),
            10: lambda: cast_big(1, "t", 1),
            11: lambda: (transpose(1, 2), load(1, "v", 1), cast_v(1, 0)),
            13: lambda: cast_v(1, 1),
        }

        state = {}

        def qk_group(b, qb, g):
            q0 = qb * QB
            qt_tp = qt_tps[b]
            s_ps = qk_psum.tile([128, 1024], F32, tag="qk")
            num = num_pool.tile([128, 1024], BF16, tag="num")
            act_spans = []      # merged contiguous spans (left-packed)
            mask_blocks = []    # span starts of diagonal blocks
            for j, c in enumerate((2 * g, 2 * g + 1)):
                i = c - 4 * qb
                lo = 128 * i if i > 0 else 0
                w = QB - lo
                ql = q0 + lo
                t0_ = ql // 128
                nt = (QB - lo) // 128
                rhs = qt_tp[:, q_slot(t0_):q_slot(t0_) + nt, :] \
                    .rearrange("p t q -> p (t q)")
                nc.tensor.matmul(
                    s_ps[:, j * 512:j * 512 + w],
                    lhsT=qt_tp[:, t_slot(c), :],
                    rhs=rhs,
                )
                if act_spans and act_spans[-1][1] == j * 512:
                    act_spans[-1] = (act_spans[-1][0], j * 512 + w)
                else:
                    act_spans.append((j * 512, j * 512 + w))
                if i >= 0:
                    mask_blocks.append(j * 512)
            for lo_, hi_ in act_spans:
                nc.scalar.activation(num[:, lo_:hi_], s_ps[:, lo_:hi_],
                                     mybir.ActivationFunctionType.Exp,
                                     scale=SCALE)
                nc.vector.tensor_scalar_max(num[:, lo_:hi_],
                                            num[:, lo_:hi_], 1.0)
            for ds in mask_blocks:
                nc.vector.tensor_tensor(num[:, ds:ds + 128],
                                        num[:, ds:ds + 128], tri01[:],
                                        op=Alu.mult)
            st = state.setdefault((b, qb), {"ob": None, "num": {}})
            if st["ob"] is None:
                st["ob"] = [ob_psum.tile([128, 2, 256], F32, tag="ob",
                                         name=f"ob_{b}_{qb}_{h}")
                            for h in range(2)]
            st["num"][g] = num

        def pv_group(b, qb, g):
            st = state[(b, qb)]
            num = st["num"].pop(g)
            v_aug = v_augs[b]
            for j, c in enumerate((2 * g, 2 * g + 1)):
                i = c - 4 * qb
                lo = 128 * i if i > 0 else 0
                for sub in range(max(i, 0), 4):
                    ob = st["ob"][sub // 2]
                    nc.tensor.matmul(
                        ob[:, sub % 2, 0:129],
                        lhsT=num[:, j * 512 + sub * 128 - lo:
                                 j * 512 + (sub + 1) * 128 - lo],
                        rhs=v_aug[:, c, 0:129],
                        start=(c == 0 and sub % 2 == 0),
                        stop=(c == 4 * qb + sub),
                        skip_group_check=True,
                    )

        def finalize(b, qb):
            st = state.pop((b, qb))
            o_tile = fin_pool.tile([128, 4, 128], F32, tag="fin")
            for h in range(2):
                ob_sb = rec_pool.tile([128, 2, 129], F32, tag="rec")
                nc.vector.tensor_copy(ob_sb[:], st["ob"][h][:, :, 0:129])
                for s2 in range(2):
                    nc.gpsimd.normalize_recip(
                        o_tile[:, 2 * h + s2, :],
                        ob_sb[:, s2, 0:128],
                        ob_sb[:, s2, 128:129])
            nc.sync.dma_start(
                o_ext[b, qb * QB:(qb + 1) * QB, :]
                    .rearrange("(s p) d -> p s d", p=128),
                o_tile[:])

        pending = deque()

        def flush_one():
            b, qb, g = pending.popleft()
            pv_group(b, qb, g)
            if g == (4 * qb + 4) // 2 - 1:
                finalize(b, qb)

        for idx, it in enumerate(items):
            qk_group(*it)
            if idx in prep_at:
                prep_at[idx]()
            pending.append(it)
            if len(pending) > 2:
                flush_one()
        while pending:
            flush_one()

    nc.compile()
    return nc


_NC_CACHE = None


def _get_nc():
    global _NC_CACHE
    if _NC_CACHE is None:
        _NC_CACHE = build_attention_core()
    return _NC_CACHE


def kernel(Q: np.ndarray, T: np.ndarray, V: np.ndarray) -> np.ndarray:
    """Full-input entry point: shard over batch, run 8-core SPMD, gather."""
    from concourse.bass_utils import run_bass_kernel_spmd

    Q = np.ascontiguousarray(np.asarray(Q, dtype=np.float32))
    T = np.ascontiguousarray(np.asarray(T, dtype=np.float32))
    V = np.ascontiguousarray(np.asarray(V, dtype=np.float32))
    assert Q.shape == (B, S, D), Q.shape

    nc = _get_nc()
    in_maps = [
        {
            "Q": Q[i * B_LOC:(i + 1) * B_LOC],
            "T": T[i * B_LOC:(i + 1) * B_LOC],
            "V": V[i * B_LOC:(i + 1) * B_LOC],
        }
        for i in range(N_CORES)
    ]
    res = run_bass_kernel_spmd(nc, in_maps, core_ids=list(range(N_CORES)))
    return np.concatenate([res.results[i]["out"] for i in range(N_CORES)], axis=0)


# revision 15
# speedup vs baseline: 2.0821x; 1.1427x over previous
"""Sparse-attention Trainium2 kernel (nn_AttentionLayer, B=16 S=2048 D=128).

reference semantics:
    A = Q @ T^T                     # [B,S,S]
    A = where(A > 0.3, A, 0)
    A += where(strictly_upper, -2^32, 0)
    y = softmax(A / sqrt(D)) @ V

Sharding: data-parallel over batch, 2 batches per core on 8 NeuronCores.

Per-core algorithm (per batch), v4:
  - Q, T cast to bf16 on VectorE into per-chunk staging tiles (separate
    tensors so the DMA xbar transposes' coarse-grained read deps don't
    serialize), transposed SBUF->SBUF by 3 large xbar calls per batch.
    Load DMAs are issued from sync/scalar/vector queues in parallel.
  - Scores computed transposed, S^T[k,q], 2 ktiles per [128,1024] PSUM
    tile. Straddling-diagonal k-tiles skip dead query columns and are
    left-packed in their PSUM bank so exp spans merge.
  - num = max(exp(S^T*scale),1): ScalarE exp (fp32 PSUM -> bf16 SBUF),
    VectorE tensor_scalar_max (4x mode). The causal mask of each
    diagonal 128x128 block is a GpSimd multiply with a 0/1 triangle.
  - PV + denominator fused per (ktile, q-subtile): lhsT = num chunk,
    rhs = [V | ones] [128k,129], PSUM-accumulated; obanks packed two
    per PSUM bank (only the bank's first matmul sets start: start=True
    arms a bank-wide lazy zero).
  - out = PV/den via a VectorE PSUM->SBUF copy + GpSimd normalize_recip.
  - PE warm-up matmuls ramp the p-state during the DMA prep; PV for
    group g is emitted two QK groups later (software pipelining).
"""

from collections import deque
from contextlib import ExitStack

import numpy as np

import concourse.bass as bass
import concourse.mybir as mybir
import concourse.tile as tile
from concourse import bacc

B, S, D = 16, 2048, 128
N_CORES = 8
B_LOC = B // N_CORES
QB = 512
KT = 128
N_QB = S // QB
N_ST = S // 128
SCALE = float(1.0 / np.sqrt(D))

F32 = mybir.dt.float32
BF16 = mybir.dt.bfloat16
Alu = mybir.AluOpType


def build_attention_core():
    nc = bacc.Bacc("TRN2", target_bir_lowering=False, debug=False,
                   num_devices=N_CORES)
    q_ext = nc.dram_tensor("Q", [B_LOC, S, D], F32, kind="ExternalInput").ap()
    t_ext = nc.dram_tensor("T", [B_LOC, S, D], F32, kind="ExternalInput").ap()
    v_ext = nc.dram_tensor("V", [B_LOC, S, D], F32, kind="ExternalInput").ap()
    o_ext = nc.dram_tensor("out", [B_LOC, S, D], F32, kind="ExternalOutput").ap()

    with tile.TileContext(nc) as tc, ExitStack() as ctx:
        const_pool = ctx.enter_context(tc.tile_pool(name="const", bufs=1))
        nat_pool = ctx.enter_context(tc.tile_pool(name="nat", bufs=1))
        stage_pool = ctx.enter_context(tc.tile_pool(name="stage", bufs=1))
        tpd_pool = ctx.enter_context(tc.tile_pool(name="tpd", bufs=1))
        vb_pool = ctx.enter_context(tc.tile_pool(name="vb", bufs=1))
        num_pool = ctx.enter_context(tc.tile_pool(name="num", bufs=6))
        fin_pool = ctx.enter_context(tc.tile_pool(name="fin", bufs=2))
        rec_pool = ctx.enter_context(tc.tile_pool(name="rec", bufs=4))
        qk_psum = ctx.enter_context(tc.tile_pool(name="qk_ps", bufs=2, space="PSUM"))
        ob_psum = ctx.enter_context(tc.tile_pool(name="ob_ps", bufs=4, space="PSUM"))

        # ---- constants (gpsimd) ----
        junk = const_pool.tile([128, 512], BF16, name="junk")
        nc.gpsimd.memset(junk[:], 0.25)
        # tri01[p, n] = 0 if p > n else 1 (first 128 cols form the in-tile
        # causal keep-mask; cols >= 128 are all ones)
        tri01 = const_pool.tile([128, 128], BF16, name="tri01")
        nc.gpsimd.memset(tri01[:], 1.0)
        nc.gpsimd.affine_select(
            out=tri01[:], in_=tri01[:],
            compare_op=Alu.is_ge, fill=0.0,
            base=0, channel_multiplier=-1, pattern=[[1, 128]])

        # ---- PE warm-up: ramp the p-state while DMA prep runs ----
        for w in range(22):
            wps = qk_psum.tile([128, 1024], F32, tag="qk", name=f"wps{w}")
            nc.tensor.matmul(wps[:, 0:512], lhsT=junk[:, 0:128], rhs=junk[:])

        # ---- staging: per batch, 3 chunk tiles (separate tensors so the
        # xbar transpose read-dep doesn't wait on later casts) ----
        # chunk c0: [q0:4 | t0:4]; chunk cQ: q4:16; chunk cT: t4:16
        nats, stages, qt_tps, v_augs = [], [], [], []
        for b in range(B_LOC):
            q_nat = nat_pool.tile([128, N_ST, D], F32, name=f"qnat{b}")
            t_nat = nat_pool.tile([128, N_ST, D], F32, name=f"tnat{b}")
            v_nat = nat_pool.tile([128, N_ST, D], F32, name=f"vnat{b}")
            ca = stage_pool.tile([128, 16, 128], BF16, name=f"ca_{b}")
            cb = stage_pool.tile([128, 16, 128], BF16, name=f"cb_{b}")
            qt_tp = tpd_pool.tile([128, 2 * N_ST, 128], BF16, name=f"qttp{b}")
            v_aug = vb_pool.tile([128, N_ST, 129], BF16, name=f"vaug{b}")
            nats.append((q_nat, t_nat, v_nat))
            stages.append((ca, cb))
            qt_tps.append(qt_tp); v_augs.append(v_aug)

        # transposed slot layout (matches [cA | cB] = [q0:8|t0:8|q8:16|t8:16]):
        def q_slot(t):
            return t if t < 8 else 8 + t

        def t_slot(c):
            return 8 + c if c < 8 else 16 + c

        def load(b, which, h):
            q_nat, t_nat, v_nat = nats[b]
            nat = {"q": q_nat, "t": t_nat, "v": v_nat}[which]
            ext = {"q": q_ext, "t": t_ext, "v": v_ext}[which]
            eng = {"q": nc.sync, "t": nc.sync, "v": nc.sync}[which]
            if which == "q" and h == 1:
                eng = nc.scalar
            if which == "t" and h == 1:
                eng = nc.scalar
            sl = slice(0, 8) if h == 0 else slice(8, 16)
            ssl = slice(0, 1024) if h == 0 else slice(1024, 2048)
            eng.dma_start(
                nat[:, sl, :],
                ext[b, ssl, :].rearrange("(t p) d -> p t d", p=128))

        def cast_chunk(b, chunk, which):
            """Cast the q- or t-half of staging chunk (A: tiles 0:8, B: 8:16)."""
            q_nat, t_nat, v_nat = nats[b]
            nat = q_nat if which == "q" else t_nat
            stg = stages[b][chunk]
            off = 0 if which == "q" else 8
            nsl = slice(0, 8) if chunk == 0 else slice(8, 16)
            nc.vector.tensor_copy(stg[:, off:off + 8, :], nat[:, nsl, :])

        def cast_v(b, h):
            q_nat, t_nat, v_nat = nats[b]
            sl = slice(0, 8) if h == 0 else slice(8, 16)
            nc.vector.memset(v_augs[b][:, sl, D:D + 1], 1.0)
            nc.vector.tensor_copy(v_augs[b][:, sl, 0:D], v_nat[:, sl, :])

        def transpose(b, chunk):
            src = stages[b][chunk]
            nc.sync.dma_start_transpose(
                qt_tps[b][:, 16 * chunk:16 * chunk + 16, :],
                src[:].rearrange("p t d -> p (t d)"))

        # ---- batch-0 head ----
        load(0, "q", 0)      # sync
        load(0, "t", 0)      # sync
        load(0, "v", 0)      # sync
        load(0, "q", 1)      # scalar queue
        load(0, "t", 1)      # scalar queue
        load(0, "v", 1)      # sync
        cast_chunk(0, 0, "q")
        cast_chunk(0, 0, "t")
        transpose(0, 0)
        cast_v(0, 0)
        cast_chunk(0, 1, "q")
        cast_chunk(0, 1, "t")
        transpose(0, 1)
        # batch-1 loads: issue now so transfers stream behind batch 0's
        load(1, "q", 0)
        load(1, "t", 0)
        load(1, "v", 0)
        load(1, "q", 1)
        load(1, "t", 1)
        load(1, "v", 1)

        items = []
        for b in range(B_LOC):
            for qb in range(N_QB):
                for g in range((4 * qb + 4) // 2):
                    items.append((b, qb, g))

        prep_at = {
            2: lambda: cast_v(0, 1),
            8: lambda: cast_chunk(1, 0, "q"),
            10: lambda: cast_chunk(1, 0, "t"),
            11: lambda: transpose(1, 0),
            12: lambda: cast_chunk(1, 1, "q"),
            14: lambda: cast_chunk(1, 1, "t"),
            15: lambda: transpose(1, 1),
            16: lambda: cast_v(1, 0),
            18: lambda: cast_v(1, 1),
        }

        state = {}

        def qk_group(b, qb, g):
            q0 = qb * QB
            qt_tp = qt_tps[b]
            s_ps = qk_psum.tile([128, 1024], F32, tag="qk")
            num = num_pool.tile([128, 1024], BF16, tag="num")
            act_spans = []      # merged contiguous spans (left-packed)
            mask_blocks = []    # span starts of diagonal blocks
            for j, c in enumerate((2 * g, 2 * g + 1)):
                i = c - 4 * qb
                lo = 128 * i if i > 0 else 0
                w = QB - lo
                ql = q0 + lo
                t0_ = ql // 128
                nt = (QB - lo) // 128
                rhs = qt_tp[:, q_slot(t0_):q_slot(t0_) + nt, :] \
                    .rearrange("p t q -> p (t q)")
                nc.tensor.matmul(
                    s_ps[:, j * 512:j * 512 + w],
                    lhsT=qt_tp[:, t_slot(c), :],
                    rhs=rhs,
                )
                if act_spans and act_spans[-1][1] == j * 512:
                    act_spans[-1] = (act_spans[-1][0], j * 512 + w)
                else:
                    act_spans.append((j * 512, j * 512 + w))
                if i >= 0:
                    mask_blocks.append(j * 512)
            for lo_, hi_ in act_spans:
                nc.scalar.activation(num[:, lo_:hi_], s_ps[:, lo_:hi_],
                                     mybir.ActivationFunctionType.Exp,
                                     scale=SCALE)
                nc.vector.tensor_scalar_max(num[:, lo_:hi_],
                                            num[:, lo_:hi_], 1.0)
            for ds in mask_blocks:
                nc.vector.tensor_tensor(num[:, ds:ds + 128],
                                        num[:, ds:ds + 128], tri01[:],
                                        op=Alu.mult)
            st = state.setdefault((b, qb), {"ob": None, "num": {}})
            if st["ob"] is None:
                st["ob"] = [ob_psum.tile([128, 2, 256], F32, tag="ob",
                                         name=f"ob_{b}_{qb}_{h}")
                            for h in range(2)]
            st["num"][g] = num

        def pv_group(b, qb, g):
            st = state[(b, qb)]
            num = st["num"].pop(g)
            v_aug = v_augs[b]
            for j, c in enumerate((2 * g, 2 * g + 1)):
                i = c - 4 * qb
                lo = 128 * i if i > 0 else 0
                for sub in range(max(i, 0), 4):
                    ob = st["ob"][sub // 2]
                    nc.tensor.matmul(
                        ob[:, sub % 2, 0:129],
                        lhsT=num[:, j * 512 + sub * 128 - lo:
                                 j * 512 + (sub + 1) * 128 - lo],
                        rhs=v_aug[:, c, 0:129],
                        start=(c == 0 and sub % 2 == 0),
                        stop=(c == 4 * qb + sub),
                        skip_group_check=True,
                    )

        def finalize(b, qb):
            st = state.pop((b, qb))
            o_tile = fin_pool.tile([128, 4, 128], F32, tag="fin")
            for h in range(2):
                ob_sb = rec_pool.tile([128, 2, 129], F32, tag="rec")
                nc.vector.tensor_copy(ob_sb[:], st["ob"][h][:, :, 0:129])
                for s2 in range(2):
                    nc.gpsimd.normalize_recip(
                        o_tile[:, 2 * h + s2, :],
                        ob_sb[:, s2, 0:128],
                        ob_sb[:, s2, 128:129])
            nc.sync.dma_start(
                o_ext[b, qb * QB:(qb + 1) * QB, :]
                    .rearrange("(s p) d -> p s d", p=128),
                o_tile[:])

        pending = deque()

        def flush_one():
            b, qb, g = pending.popleft()
            pv_group(b, qb, g)
            if g == (4 * qb + 4) // 2 - 1:
                finalize(b, qb)

        for idx, it in enumerate(items):
            qk_group(*it)
            if idx in prep_at:
                prep_at[idx]()
            pending.append(it)
            if len(pending) > 2:
                flush_one()
        while pending:
            flush_one()

    nc.compile()
    return nc


_NC_CACHE = None


def _get_nc():
    global _NC_CACHE
    if _NC_CACHE is None:
        _NC_CACHE = build_attention_core()
    return _NC_CACHE


def kernel(Q: np.ndarray, T: np.ndarray, V: np.ndarray) -> np.ndarray:
    """Full-input entry point: shard over batch, run 8-core SPMD, gather."""
    from concourse.bass_utils import run_bass_kernel_spmd

    Q = np.ascontiguousarray(np.asarray(Q, dtype=np.float32))
    T = np.ascontiguousarray(np.asarray(T, dtype=np.float32))
    V = np.ascontiguousarray(np.asarray(V, dtype=np.float32))
    assert Q.shape == (B, S, D), Q.shape

    nc = _get_nc()
    in_maps = [
        {
            "Q": Q[i * B_LOC:(i + 1) * B_LOC],
            "T": T[i * B_LOC:(i + 1) * B_LOC],
            "V": V[i * B_LOC:(i + 1) * B_LOC],
        }
        for i in range(N_CORES)
    ]
    res = run_bass_kernel_spmd(nc, in_maps, core_ids=list(range(N_CORES)))
    return np.concatenate([res.results[i]["out"] for i in range(N_CORES)], axis=0)
